# revision 1
# baseline (speedup 1.0000x reference)
"""Distributed Trainium2 Bass kernel for perceiver-style cross-attention.

Reference computation (per batch element b of 64):
    query = q[b] @ Wq                      # (128, 1024)
    k, v  = split(kv[b] @ Wkv, 2)          # (512, 1024) each
    per head h (16 heads, dim 64):
        S_h = (q_h @ k_h^T) / 8            # (128, 512)
        P_h = softmax(S_h, axis=-1)
        O_h = P_h @ v_h                    # (128, 64)
    out[b] = concat_h(O_h) @ Wo + bo       # (128, 512)

Sharding: pure data-parallel over the 64-asset batch axis -> 8 assets per
NeuronCore, no collectives.

Design (v4):
  - q and kv are transposed on the HOST (numpy): no on-chip transposes.
  - Everything is bf16 (inputs, weights, intermediates); accumulation stays
    fp32 in PSUM.  Verified offline: rel err ~5e-3 vs the f32 reference.
  - Attention is emitted in HEAD-PAIR slots: one 2-bank PSUM score tile per
    head pair (the natural kT/queryT head-pair layout), ONE exp per pair
    (halves ScalarE instruction overhead - ScalarE paces attention), and
    an 8-matmul PV block into a 4-head aug tile.
  - The exp-paced slots leave the PE idle ~50% of each slot, so the V/K
    projections of asset a+1 ride along as per-slot PE filler (V halves in
    slots 0-3, K head-pairs in slots 4-7) and the final projection of
    asset a-1 runs between assets.  Eviction copies that land on ScalarE
    are deferred until after the slot's exp is issued so they can never
    delay the critical exp.
  - Scores are computed transposed, scoresT[j, i]: lhsT = kT slice,
    rhs = queryT slice (bf16, K=64).
  - exp(x/8) straight out of PSUM into bf16; no max subtraction
    (|scores|/8 < 8 for this problem's data, verified offline).
  - PV uses v natural with a ones column appended, so the softmax
    denominators drop out of the same matmul (row 64 of the PSUM tile).
  - Normalization: reciprocal_approx_fast on DVE DIRECTLY from the PSUM s
    row (~51 ULP, plenty for a softmax denominator), GPSIMD broadcast,
    three DVE multiplies - no SBUF round-trip DMAs in the chain.
  - Output bias is added during the final-projection eviction (DVE
    tensor_add against a GPSIMD-pre-broadcast bias plane).
  - Phase 0 (Q projection) streams cc-major over a 6-bank transient pool
    in three blocks, with asset 0's V/K projections interleaved between
    blocks; input DMAs are emitted in consumption order (each DMA costs
    ~625ns of serial HWDGE time, so chunks are consumption-granular).
"""

import sys
import numpy as np

for _p in ("/opt/trn_rl_repo", "/opt/pypackages"):
    if _p not in sys.path:
        sys.path.append(_p)

from contextlib import ExitStack

import concourse.bass as bass  # noqa: E402
import concourse.tile as tile  # noqa: E402
from concourse import bacc, mybir  # noqa: E402

F32 = mybir.dt.float32
BF16 = mybir.dt.bfloat16

N_CORES = 8
B_LOC = 8  # assets per core
I = 128  # num_latents
J = 512  # window size
QD = 512  # q feature dim
KVD = 256  # kv feature dim
H = 16  # heads
D = 64  # head dim
HID = 1024  # H * D
NO = 512  # output dim


def build_nc():
    nc = bacc.Bacc(
        "TRN2", target_bir_lowering=False, debug=False, num_devices=N_CORES
    )

    qT_ext = nc.dram_tensor("qT", [QD, B_LOC * I], BF16, kind="ExternalInput").ap()
    kvT_ext = nc.dram_tensor("kvT", [B_LOC, KVD, J], BF16, kind="ExternalInput").ap()
    wq_ext = nc.dram_tensor("Wq", [QD, HID], BF16, kind="ExternalInput").ap()
    wkv_ext = nc.dram_tensor("Wkv", [KVD, 2 * HID], BF16, kind="ExternalInput").ap()
    wo_ext = nc.dram_tensor("Wo", [HID, NO], BF16, kind="ExternalInput").ap()
    bo_ext = nc.dram_tensor("bo", [NO], F32, kind="ExternalInput").ap()
    out_ext = nc.dram_tensor("out", [B_LOC, I, NO], BF16, kind="ExternalOutput").ap()

    with tile.TileContext(nc) as tc, ExitStack() as ctx:
        consts = ctx.enter_context(tc.tile_pool(name="consts", bufs=1))

        wq_sb = consts.tile([128, 4, HID], BF16, name="wq")
        wkv_sb = consts.tile([128, 2, 2 * HID], BF16, name="wkv")
        wo_sb = consts.tile([128, 8, NO], BF16, name="wo")
        bo_row = consts.tile([1, NO], F32, name="bo_row")
        bo_bc = consts.tile([128, NO], F32, name="bo_bc")
        qT_sb = consts.tile([128, 4, B_LOC * I], BF16, name="qT_sb")
        kvT_sb = [
            consts.tile([128, 2, J], BF16, name=f"kvT{a}") for a in range(B_LOC)
        ]

        # Input DMAs in consumption order.  Sync queue: (qT, wq-h0) chunk
        # pairs for the cc-major Q projection, then the wq h1 halves.
        # Scalar queue: asset 0's kv first (needed mid-phase-0), weights,
        # remaining kv, output weights.
        # Sync queue: qT per-cc (the cc-major Q projection consumes one
        # chunk at a time; HWDGE generation is ~625ns serial per DMA, the
        # transfers themselves overlap).  Scalar queue, in deadline order:
        # the two wq halves as single DMAs, asset 0's kv, wkv, the rest.
        # DMA queues have no exec-queue depth (each DMA occupies its queue
        # for ~2us of latency), so the input set is spread across all four
        # queues in per-queue consumption order.
        def _wqh0(cc):
            return (wq_sb[:, cc, 0:512], wq_ext[cc * 128 : (cc + 1) * 128, 0:512])

        def _wqh1(cc):
            return (
                wq_sb[:, cc, 512:1024],
                wq_ext[cc * 128 : (cc + 1) * 128, 512:1024],
            )

        def _qt(cc):
            return (qT_sb[:, cc, :], qT_ext[cc * 128 : (cc + 1) * 128, :])

        def _kvt(a):
            return (kvT_sb[a], kvT_ext[a].rearrange("(cc p) j -> p cc j", p=128))

        def _wkv(cc):
            return (wkv_sb[:, cc, :], wkv_ext[cc * 128 : (cc + 1) * 128, :])

        # per-queue lists, each in its own consumption order; the three
        # queues issue concurrently (~2us serial latency per DMA per queue).
        # Keep big-descriptor transfers off the gpsimd/SWDGE queue (its
        # descriptor ring holds only 1024 entries).
        # The scalar (ScalarE) queue gets ONLY the early chunks: every DMA
        # dispatch occupies that queue ~0.9us and would otherwise delay the
        # phase-0 evictions that run on ScalarE.  Everything mid/late rides
        # the sync queue, which is idle after phase 0.
        wo_re = wo_ext.rearrange("(c p) no -> p c no", p=128)
        sync_q = [
            _wqh0(0), _wqh0(1), _qt(2), _qt(3), _wqh1(2),
            _kvt(1), _kvt(3), _kvt(4), _kvt(6), _kvt(7),
            (wo_sb[:, 0:4, :], wo_re[:, 0:4, :]),
            (wo_sb[:, 4:8, :], wo_re[:, 4:8, :]),
        ]
        scalar_q = [_qt(0), _wqh0(2), _wqh0(3), _wkv(1), _wqh1(1)]
        gpsimd_q = [
            _qt(1),
            _wkv(0),
            _kvt(0),
            _wqh1(0),
            _wqh1(3),
            (bo_row, bo_ext.unsqueeze(0)),
            _kvt(2),
            _kvt(5),
        ]
        for dst, src in sync_q:
            nc.sync.dma_start(dst, src)
        for dst, src in scalar_q:
            nc.scalar.dma_start(dst, src)
        for dst, src in gpsimd_q:
            nc.gpsimd.dma_start(dst, src)
        nc.gpsimd.partition_broadcast(bo_bc, bo_row)

        # Persistent pools.
        queryT_pool = ctx.enter_context(tc.tile_pool(name="queryTp", bufs=1))
        l_pool = ctx.enter_context(tc.tile_pool(name="lp", bufs=1))

        # queryT: one [128, B*I] bf16 tile per head-PAIR (2 heads stacked on
        # partitions; base-64 operand slices are legal, HW-verified).
        queryT = [
            queryT_pool.tile([128, B_LOC * I], BF16, name=f"queryT{hc}")
            for hc in range(8)
        ]
        # normalized out^T chunks, 2 per asset -> 16, consumed by the final
        # projection one asset later
        lgs = [
            [
                l_pool.tile([128, 4, I], BF16, name=f"lg{a}_{g}", tag=f"lg{a}_{g}")
                for g in range(2)
            ]
            for a in range(B_LOC)
        ]

        # PSUM layout: proj pool FIRST (banks 0-1) so asset 0's projections
        # can run while the transient phase-0 pool (banks 2-7) is live.
        proj_ps_pool = ctx.enter_context(
            tc.tile_pool(name="proj_ps", bufs=2, space="PSUM")
        )

        kT_pool = ctx.enter_context(tc.tile_pool(name="kTp", bufs=2))
        v_pool = ctx.enter_context(tc.tile_pool(name="vp", bufs=2))
        exp_pool = ctx.enter_context(tc.tile_pool(name="expp", bufs=6))
        s_pool = ctx.enter_context(tc.tile_pool(name="sp", bufs=8))
        rb_pool = ctx.enter_context(tc.tile_pool(name="rbp", bufs=6))
        o_pool = ctx.enter_context(tc.tile_pool(name="op", bufs=2))

        kT_tiles = {}
        vaug_tiles = {}

        def emit_vaug_alloc(a):
            vaug = v_pool.tile([128, 4, H, D + 1], BF16, name="vaug", tag="vaug")
            if a == 0:
                # phase 0: the gpsimd queue is busy with input DMAs
                nc.vector.memset(vaug[:, :, :, D : D + 1], 1.0)
            else:
                # steady state: memset on the otherwise-idle GPSIMD engine
                nc.gpsimd.memset(vaug[:, :, :, D : D + 1], 1.0)
            vaug_tiles[a] = vaug

        def emit_vproj_half(a, jc, nh):
            # one (j-chunk, nh) half of the V projection: 2 matmuls, 1 bank
            kvt = kvT_sb[a]
            vaug = vaug_tiles[a]
            ps = proj_ps_pool.tile([128, 512], F32, name="vps", tag="pps")
            for cc in range(2):
                nc.tensor.matmul(
                    ps,
                    kvt[:, cc, jc * 128 : (jc + 1) * 128],
                    wkv_sb[:, cc, HID + nh * 512 : HID + (nh + 1) * 512],
                    start=(cc == 0),
                    stop=(cc == 1),
                )
            if nh == 1:
                # one ScalarE eviction per slot, deferred past the exp
                def evict():
                    nc.scalar.copy(
                        vaug[:, jc, nh * 8 : (nh + 1) * 8, 0:D],
                        ps.rearrange("p (h d) -> p h d", h=8),
                    )

                return evict
            nc.vector.tensor_copy(
                vaug[:, jc, nh * 8 : (nh + 1) * 8, 0:D],
                ps.rearrange("p (h d) -> p h d", h=8),
            )
            return None  # eviction on DVE never delays the exp

        def emit_kproj_block(a, hc):
            # one head-pair of the K projection: 2 matmuls; the ScalarE
            # evictions are returned for post-exp deferral
            kvt = kvT_sb[a]
            if hc == 0:
                kT_tiles[a] = kT_pool.tile([128, 8, J], BF16, name="kT", tag="kT")
            kT = kT_tiles[a]
            ps = proj_ps_pool.tile([128, 512], F32, name="kps", tag="pps")
            for cc in range(2):
                nc.tensor.matmul(
                    ps,
                    wkv_sb[:, cc, hc * 128 : (hc + 1) * 128],
                    kvt[:, cc, :],
                    start=(cc == 0),
                    stop=(cc == 1),
                )
            if hc % 2 == 0 or hc == 3:
                # K3 rides slot t7: its eviction must free the proj bank
                # before slot t0's V block, so it can't be exp-deferred
                nc.vector.tensor_copy(kT[:, hc, :], ps)
                return None

            def evict():
                nc.scalar.copy(kT[:, hc, :], ps)

            return evict

        def emit_final(a):
            fps = proj_ps_pool.tile([128, NO], F32, name="pps", tag="pps")
            for cc in range(4):
                nc.tensor.matmul(
                    fps, lgs[a][0][:, cc, :], wo_sb[:, cc, :],
                    start=(cc == 0), stop=False,
                )
            for cc in range(4):
                nc.tensor.matmul(
                    fps, lgs[a][1][:, cc, :], wo_sb[:, 4 + cc, :],
                    start=False, stop=(cc == 3),
                )
            out_sb = o_pool.tile([128, NO], BF16, name="out_sb", tag="out_sb")
            nc.vector.tensor_add(out_sb, fps, bo_bc)
            nc.sync.dma_start(out_ext[a], out_sb)

        # ---------------- phase 0: Q projection --------------------------
        # Three hc-blocks (3+3+2) cc-major over a 6-bank pool; both nh
        # halves share each stationary (LDWEIGHTS halved).  Asset 0's V/K
        # projection blocks are interleaved between phase-0 blocks (they
        # use the separate 2-bank proj pool).
        ph0 = ExitStack()
        qps_pool = ph0.enter_context(tc.tile_pool(name="qps", bufs=1, space="PSUM"))

        def qproj_block(hcs):
            ps = [
                [
                    qps_pool.tile(
                        [128, 512], F32, name=f"qps{x}{k}", tag=f"qps{x}{k}"
                    )
                    for k in range(len(hcs))
                ]
                for x in range(2)
            ]
            for cc in range(4):
                for k, hc in enumerate(hcs):
                    st = wq_sb[:, cc, hc * 128 : (hc + 1) * 128]
                    nc.tensor.matmul(
                        ps[0][k], st, qT_sb[:, cc, 0:512],
                        start=(cc == 0), stop=(cc == 3),
                    )
                    nc.tensor.matmul(
                        ps[1][k], st, qT_sb[:, cc, 512:1024],
                        start=(cc == 0), stop=(cc == 3),
                    )
            for k, hc in enumerate(hcs):
                if k % 2 == 0:
                    nc.vector.tensor_copy(queryT[hc][:, 0:512], ps[0][k])
                    nc.scalar.copy(queryT[hc][:, 512:1024], ps[1][k])
                else:
                    nc.scalar.copy(queryT[hc][:, 0:512], ps[0][k])
                    nc.vector.tensor_copy(queryT[hc][:, 512:1024], ps[1][k])

        qproj_block([0, 1, 2])
        emit_vaug_alloc(0)
        for jc in range(4):
            for nh in range(2):
                ev = emit_vproj_half(0, jc, nh)
                if ev is not None:
                    ev()
        qproj_block([3, 4, 5])
        for hc in range(4):
            ev = emit_kproj_block(0, hc)
            if ev is not None:
                ev()
        qproj_block([6, 7])
        for hc in range(4, 8):
            ev = emit_kproj_block(0, hc)
            if ev is not None:
                ev()
        ph0.close()

        # attention PSUM pools (reuse the phase-0 banks)
        score_ps_pool = ctx.enter_context(
            tc.tile_pool(name="score_ps", bufs=2, space="PSUM")
        )
        aug_ps_pool = ctx.enter_context(
            tc.tile_pool(name="aug_ps", bufs=2, space="PSUM")
        )

        def emit_scores2(a, t):
            # head pair (2t, 2t+1): head-pair tile hc == t, hp == head % 2
            kT = kT_tiles[a]
            sps = score_ps_pool.tile([128, 2, 4, I], F32, name="sps", tag="sps")
            for e in range(2):
                for jc in range(4):
                    nc.tensor.matmul(
                        sps[:, e, jc, :],
                        kT[e * D : (e + 1) * D, t, jc * 128 : (jc + 1) * 128],
                        queryT[t][e * D : (e + 1) * D, a * I : (a + 1) * I],
                        start=True,
                        stop=True,
                    )
            return sps

        def emit_exp_pv2(a, t, sps, aug):
            vaug = vaug_tiles[a]
            expT = exp_pool.tile([128, 2, 4, I], BF16, name="expT", tag="expT")
            nc.scalar.activation(
                expT,
                sps,
                mybir.ActivationFunctionType.Exp,
                bias=0.0,
                scale=0.125,
            )
            for e in range(2):
                h = 2 * t + e
                for jc in range(4):
                    nc.tensor.matmul(
                        aug[:, 2 * (t % 2) + e, :],
                        vaug[:, jc, h, :],
                        expT[:, e, jc, :],
                        start=(jc == 0),
                        stop=(jc == 3),
                    )

        def emit_normalize(a, hg, aug):
            # 1/s: ScalarE copies the PSUM s row (partition 64) down to
            # partition 0, approx reciprocal on DVE (custom uops want base
            # partition 0), broadcast across partitions, three multiplies
            srow = s_pool.tile([1, 4 * I], F32, name="srow", tag="srow")
            if hg % 2 == 0:
                nc.scalar.copy(srow, aug[D : D + 1, :, :])
            else:
                nc.vector.tensor_copy(srow, aug[D : D + 1, :, :])
            rrow = s_pool.tile([1, 4 * I], F32, name="rrow", tag="rrow")
            nc.vector.reciprocal_approx_fast(rrow, srow)
            rb = rb_pool.tile([128, 4, I], F32, name="rb", tag="rb")
            nc.gpsimd.partition_broadcast(rb[:], rrow[:])

            g, half = hg // 2, hg % 2
            lg = lgs[a][g][:, 2 * half : 2 * half + 2, :]
            nc.vector.tensor_mul(
                lg[0:64, :, :], aug[0:64, 0:4:2, :], rb[0:64, 0:4:2, :]
            )
            nc.vector.tensor_mul(
                lg[64:128, :, :], aug[0:64, 1:4:2, :], rb[0:64, 1:4:2, :]
            )

        # Main loop: one continuous head-pair-slot pipeline across ALL
        # assets (the exp of slot s is issued after the scores of slot s+1,
        # including across asset boundaries, so no exp latency is ever
        # exposed).  Projections for asset a+1 and the final projection of
        # asset a-1 are spread across the slots as PE filler:
        #   t0: vaug alloc + V(jc0) halves       t4: V(jc2) halves
        #   t1: K(a, hc4-5)                      t5: V(jc3) halves + K(x, 0)
        #   t2: K(a, hc6-7)                      t6: K(x, 1-2)
        #   t3: V(jc1) halves                    t7: K(x, 3) + final(a-1)
        # (x = a+1; K blocks for x continue into x's own slots t1-t2.)
        def filler_list(a, t):
            x = a + 1
            fl = []
            if t == 0 and x < B_LOC:
                fl = [
                    lambda: emit_vaug_alloc(x),
                    lambda: emit_vproj_half(x, 0, 0),
                    lambda: emit_vproj_half(x, 0, 1),
                ]
            elif t == 1 and a >= 1:
                fl = [
                    lambda: emit_kproj_block(a, 4),
                    lambda: emit_kproj_block(a, 5),
                ]
            elif t == 2 and a >= 1:
                fl = [
                    lambda: emit_kproj_block(a, 6),
                    lambda: emit_kproj_block(a, 7),
                ]
            elif t in (3, 4) and x < B_LOC:
                jc = t - 2
                fl = [
                    (lambda: emit_vproj_half(x, jc, 0)),
                    (lambda: emit_vproj_half(x, jc, 1)),
                ]
            elif t == 5 and x < B_LOC:
                fl = [
                    lambda: emit_vproj_half(x, 3, 0),
                    lambda: emit_vproj_half(x, 3, 1),
                    lambda: emit_kproj_block(x, 0),
                ]
            elif t == 6 and x < B_LOC:
                fl = [
                    lambda: emit_kproj_block(x, 1),
                    lambda: emit_kproj_block(x, 2),
                ]
            elif t == 7:
                fl = []
                if x < B_LOC:
                    fl.append(lambda: emit_kproj_block(x, 3))
                if a >= 1:
                    fl.append(lambda: emit_final(a - 1))
            return fl

        augs = {}

        def get_aug(a, t):
            hg = t // 2
            if (a, hg) not in augs:
                augs[(a, hg)] = aug_ps_pool.tile(
                    [D + 1, 4, I], F32, name="aug", tag="aug"
                )
            return augs[(a, hg)]

        prev = None
        deferred = []
        for a in range(B_LOC):
            for t in range(8):
                cur = emit_scores2(a, t)
                evs = []
                for f in filler_list(a, t):
                    ev = f()
                    if ev is not None:
                        evs.append(ev)
                if prev is not None:
                    pa, pt, psps = prev
                    emit_exp_pv2(pa, pt, psps, get_aug(pa, pt))
                    for ev in deferred:
                        ev()
                    if pt % 2 == 1:
                        emit_normalize(pa, pt // 2, augs.pop((pa, pt // 2)))
                deferred = evs
                prev = (a, t, cur)
        pa, pt, psps = prev
        emit_exp_pv2(pa, pt, psps, get_aug(pa, pt))
        for ev in deferred:
            ev()
        emit_normalize(pa, 3, augs.pop((pa, 3)))
        emit_final(B_LOC - 1)

    nc.compile()
    return nc


def make_in_maps(q, kv, Wq, Wkv, Wo, bo):
    import ml_dtypes

    bf = ml_dtypes.bfloat16
    q = np.asarray(q, dtype=np.float32).astype(bf)
    kv = np.asarray(kv, dtype=np.float32).astype(bf)
    Wqb = np.ascontiguousarray(np.asarray(Wq, dtype=np.float32).astype(bf))
    Wkvb = np.ascontiguousarray(np.asarray(Wkv, dtype=np.float32).astype(bf))
    Wob = np.ascontiguousarray(np.asarray(Wo, dtype=np.float32).astype(bf))
    bo = np.ascontiguousarray(np.asarray(bo, dtype=np.float32))

    in_maps = []
    for c in range(N_CORES):
        sl = slice(c * B_LOC, (c + 1) * B_LOC)
        qT = np.ascontiguousarray(
            q[sl].transpose(2, 0, 1).reshape(QD, B_LOC * I)
        )
        kvT = np.ascontiguousarray(kv[sl].transpose(0, 2, 1))
        in_maps.append(
            {"qT": qT, "kvT": kvT, "Wq": Wqb, "Wkv": Wkvb, "Wo": Wob, "bo": bo}
        )
    return in_maps


_CACHED_NC = None


def kernel(q, kv, Wq, Wkv, Wo, bo):
    global _CACHED_NC
    from concourse.bass_utils import run_bass_kernel_spmd

    if _CACHED_NC is None:
        _CACHED_NC = build_nc()
    nc = _CACHED_NC

    in_maps = make_in_maps(q, kv, Wq, Wkv, Wo, bo)
    res = run_bass_kernel_spmd(nc, in_maps, list(range(N_CORES)))
    out = np.concatenate(
        [
            np.asarray(res.results[c]["out"], dtype=np.float32).reshape(
                B_LOC, I, NO
            )
            for c in range(N_CORES)
        ],
        axis=0,
    )
    return out



# revision 5
# speedup vs baseline: 1.1571x; 1.1571x over previous
"""Distributed Trainium2 Bass kernel for perceiver-style cross-attention.

Reference computation (per batch element b of 64):
    query = q[b] @ Wq                      # (128, 1024)
    k, v  = split(kv[b] @ Wkv, 2)          # (512, 1024) each
    per head h (16 heads, dim 64):
        S_h = (q_h @ k_h^T) / 8            # (128, 512)
        P_h = softmax(S_h, axis=-1)
        O_h = P_h @ v_h                    # (128, 64)
    out[b] = concat_h(O_h) @ Wo + bo       # (128, 512)

Sharding (v5): ALL 64 assets on ONE NeuronCore.  The per-call cost of this
problem is dominated by per-device NEFF dispatch overhead (~1.25 ms/device
through the axon PJRT shard_map path), not by device execution (~160 us per
8 assets).  Running 8 cores costs 8 dispatch units for 0.16 ms of hidden
compute; running 1 core costs 1 dispatch unit + ~1.3 ms of compute - a
~4x lower per-call total.  The 64 assets are processed as 8 GROUPS of 8,
with the group g+1 input DMAs prefetched during group g's compute.

Per-group design (v4, unchanged):
  - q and kv are transposed on the HOST (numpy): no on-chip transposes.
  - Everything is bf16 (inputs, weights, intermediates); accumulation stays
    fp32 in PSUM.  Verified: rel err ~5e-3 vs the f32 reference.
  - Attention is emitted in HEAD-PAIR slots: one 2-bank PSUM score tile per
    head pair, ONE exp per pair (ScalarE paces attention), and an 8-matmul
    PV block into a 4-head aug tile.
  - The exp-paced slots leave the PE idle ~50% of each slot, so the V/K
    projections of asset a+1 ride along as per-slot PE filler and the final
    projection of asset a-1 runs between assets.  ScalarE eviction copies
    are deferred until after the slot's exp is issued.
  - Scores are computed transposed, scoresT[j, i]: lhsT = kT slice,
    rhs = queryT slice (bf16, K=64).
  - exp(x/8) straight out of PSUM into bf16; no max subtraction
    (|scores|/8 < 8 for this problem's data, verified offline).
  - PV uses v natural with a ones column appended, so the softmax
    denominators drop out of the same matmul (row 64 of the PSUM tile).
  - Normalization: reciprocal_approx_fast on DVE directly from the PSUM s
    row, GPSIMD broadcast, three DVE multiplies.
  - Output bias is added during the final-projection eviction.
  - Phase 0 (per-group Q projection) streams cc-major over a 6-bank
    transient pool, with the group-first asset's V/K projections
    interleaved between blocks.
"""

import sys
import numpy as np

for _p in ("/opt/trn_rl_repo", "/opt/pypackages"):
    if _p not in sys.path:
        sys.path.append(_p)

from contextlib import ExitStack

import concourse.bass as bass  # noqa: E402
import concourse.tile as tile  # noqa: E402
from concourse import bacc, mybir  # noqa: E402

F32 = mybir.dt.float32
BF16 = mybir.dt.bfloat16

N_CORES = 1
TOT = 64 // N_CORES  # assets per core
B_G = 8  # assets per group
G = TOT // B_G  # groups per core
I = 128  # num_latents
J = 512  # window size
QD = 512  # q feature dim
KVD = 256  # kv feature dim
H = 16  # heads
D = 64  # head dim
HID = 1024  # H * D
NO = 512  # output dim
GW = B_G * I  # columns per group in qT / queryT (1024)


def build_nc():
    nc = bacc.Bacc(
        "TRN2", target_bir_lowering=False, debug=False, num_devices=N_CORES
    )

    qT_ext = nc.dram_tensor("qT", [QD, TOT * I], BF16, kind="ExternalInput").ap()
    kvT_ext = nc.dram_tensor("kvT", [TOT, KVD, J], BF16, kind="ExternalInput").ap()
    wq_ext = nc.dram_tensor("Wq", [QD, HID], BF16, kind="ExternalInput").ap()
    wkv_ext = nc.dram_tensor("Wkv", [KVD, 2 * HID], BF16, kind="ExternalInput").ap()
    wo_ext = nc.dram_tensor("Wo", [HID, NO], BF16, kind="ExternalInput").ap()
    bo_ext = nc.dram_tensor("bo", [NO], F32, kind="ExternalInput").ap()
    out_ext = nc.dram_tensor("out", [TOT, I, NO], BF16, kind="ExternalOutput").ap()

    with tile.TileContext(nc) as tc, ExitStack() as ctx:
        consts = ctx.enter_context(tc.tile_pool(name="consts", bufs=1))

        wq_sb = consts.tile([128, 4, HID], BF16, name="wq")
        wkv_sb = consts.tile([128, 2, 2 * HID], BF16, name="wkv")
        wo_sb = consts.tile([128, 8, NO], BF16, name="wo")
        bo_row = consts.tile([1, NO], F32, name="bo_row")
        bo_bc = consts.tile([128, NO], F32, name="bo_bc")

        # Group-level input/activation tiles, double-buffered: group g+1 is
        # prefetched while group g computes.
        grp_pool = ctx.enter_context(tc.tile_pool(name="grp", bufs=2))
        qTg_t = {}  # g -> [128, 4, GW] bf16
        kvTg_t = {}  # g -> [128, B_G, 2, J] bf16
        queryTg_t = {}  # g -> [128, 8, GW] bf16 (head-pair-major)

        def alloc_group_in(g):
            qTg_t[g] = grp_pool.tile([128, 4, GW], BF16, name="qTg", tag="qTg")
            kvTg_t[g] = grp_pool.tile(
                [128, B_G, 2, J], BF16, name="kvTg", tag="kvTg"
            )

        # ---- group 0 input DMAs, in consumption order ------------------
        # Sync queue: (qT, wq-h0) chunk pairs for the cc-major Q projection.
        # Scalar queue gets ONLY early chunks (DMA dispatch occupies it
        # ~0.9us and would delay phase-0 evictions).  Everything mid/late
        # rides sync/gpsimd.
        alloc_group_in(0)

        def _wqh0(cc):
            return (wq_sb[:, cc, 0:512], wq_ext[cc * 128 : (cc + 1) * 128, 0:512])

        def _wqh1(cc):
            return (
                wq_sb[:, cc, 512:1024],
                wq_ext[cc * 128 : (cc + 1) * 128, 512:1024],
            )

        def _qt(cc):
            return (qTg_t[0][:, cc], qT_ext[cc * 128 : (cc + 1) * 128, 0:GW])

        def _kvt(a):
            return (
                kvTg_t[0][:, a],
                kvT_ext[a].rearrange("(cc p) j -> p cc j", p=128),
            )

        def _wkv(cc):
            return (wkv_sb[:, cc, :], wkv_ext[cc * 128 : (cc + 1) * 128, :])

        wo_re = wo_ext.rearrange("(c p) no -> p c no", p=128)
        sync_q = [
            _wqh0(0), _wqh0(1), _qt(2), _qt(3), _wqh1(2),
            _kvt(1), _kvt(3), _kvt(4), _kvt(6), _kvt(7),
            (wo_sb[:, 0:4, :], wo_re[:, 0:4, :]),
            (wo_sb[:, 4:8, :], wo_re[:, 4:8, :]),
        ]
        scalar_q = [_qt(0), _wqh0(2), _wqh0(3), _wkv(1), _wqh1(1)]
        gpsimd_q = [
            _qt(1),
            _wkv(0),
            _kvt(0),
            _wqh1(0),
            _wqh1(3),
            (bo_row, bo_ext.unsqueeze(0)),
            _kvt(2),
            _kvt(5),
        ]
        for dst, src in sync_q:
            nc.sync.dma_start(dst, src)
        for dst, src in scalar_q:
            nc.scalar.dma_start(dst, src)
        for dst, src in gpsimd_q:
            nc.gpsimd.dma_start(dst, src)
        nc.gpsimd.partition_broadcast(bo_bc, bo_row)

        def prefetch_group(g):
            # input DMAs for group g, issued a full group (~150us) ahead on
            # the two least-loaded queues (ScalarE paces attention).
            alloc_group_in(g)
            qt, kt = qTg_t[g], kvTg_t[g]
            for cc in range(4):
                eng = nc.sync if cc % 2 == 0 else nc.gpsimd
                eng.dma_start(
                    qt[:, cc],
                    qT_ext[cc * 128 : (cc + 1) * 128, g * GW : (g + 1) * GW],
                )
            for ai in range(B_G):
                a = g * B_G + ai
                eng = nc.sync if ai % 2 == 0 else nc.gpsimd
                eng.dma_start(
                    kt[:, ai], kvT_ext[a].rearrange("(cc p) j -> p cc j", p=128)
                )

        # Persistent per-asset pools.
        l_pool = ctx.enter_context(tc.tile_pool(name="lp", bufs=2))
        proj_ps_pool = ctx.enter_context(
            tc.tile_pool(name="proj_ps", bufs=2, space="PSUM")
        )
        kT_pool = ctx.enter_context(tc.tile_pool(name="kTp", bufs=2))
        v_pool = ctx.enter_context(tc.tile_pool(name="vp", bufs=2))
        exp_pool = ctx.enter_context(tc.tile_pool(name="expp", bufs=6))
        s_pool = ctx.enter_context(tc.tile_pool(name="sp", bufs=8))
        rb_pool = ctx.enter_context(tc.tile_pool(name="rbp", bufs=6))
        o_pool = ctx.enter_context(tc.tile_pool(name="op", bufs=2))

        kT_tiles = {}
        vaug_tiles = {}
        lgs = {}  # a -> [lg_tile_g0, lg_tile_g1]

        def emit_vaug_alloc(a):
            vaug = v_pool.tile([128, 4, H, D + 1], BF16, name="vaug", tag="vaug")
            if a == 0:
                # group-0 phase 0: the gpsimd queue is busy with input DMAs
                nc.vector.memset(vaug[:, :, :, D : D + 1], 1.0)
            else:
                nc.gpsimd.memset(vaug[:, :, :, D : D + 1], 1.0)
            vaug_tiles[a] = vaug

        def emit_vproj_half(a, jc, nh):
            # one (j-chunk, nh) half of the V projection: 2 matmuls, 1 bank
            kvt = kvTg_t[a // B_G][:, a % B_G]
            vaug = vaug_tiles[a]
            ps = proj_ps_pool.tile([128, 512], F32, name="vps", tag="pps")
            for cc in range(2):
                nc.tensor.matmul(
                    ps,
                    kvt[:, cc, jc * 128 : (jc + 1) * 128],
                    wkv_sb[:, cc, HID + nh * 512 : HID + (nh + 1) * 512],
                    start=(cc == 0),
                    stop=(cc == 1),
                )
            if nh == 1:
                # one ScalarE eviction per slot, deferred past the exp
                def evict():
                    nc.scalar.copy(
                        vaug[:, jc, nh * 8 : (nh + 1) * 8, 0:D],
                        ps.rearrange("p (h d) -> p h d", h=8),
                    )

                return evict
            nc.vector.tensor_copy(
                vaug[:, jc, nh * 8 : (nh + 1) * 8, 0:D],
                ps.rearrange("p (h d) -> p h d", h=8),
            )
            return None  # eviction on DVE never delays the exp

        def emit_kproj_block(a, hc):
            # one head-pair of the K projection: 2 matmuls; the ScalarE
            # evictions are returned for post-exp deferral
            kvt = kvTg_t[a // B_G][:, a % B_G]
            if hc == 0:
                kT_tiles[a] = kT_pool.tile([128, 8, J], BF16, name="kT", tag="kT")
            kT = kT_tiles[a]
            ps = proj_ps_pool.tile([128, 512], F32, name="kps", tag="pps")
            for cc in range(2):
                nc.tensor.matmul(
                    ps,
                    wkv_sb[:, cc, hc * 128 : (hc + 1) * 128],
                    kvt[:, cc, :],
                    start=(cc == 0),
                    stop=(cc == 1),
                )
            if hc % 2 == 0 or hc == 3:
                # K3 rides slot t7: its eviction must free the proj bank
                # before slot t0's V block, so it can't be exp-deferred
                nc.vector.tensor_copy(kT[:, hc, :], ps)
                return None

            def evict():
                nc.scalar.copy(kT[:, hc, :], ps)

            return evict

        def emit_final(a):
            lg = lgs.pop(a)
            fps = proj_ps_pool.tile([128, NO], F32, name="pps", tag="pps")
            for cc in range(4):
                nc.tensor.matmul(
                    fps, lg[0][:, cc, :], wo_sb[:, cc, :],
                    start=(cc == 0), stop=False,
                )
            for cc in range(4):
                nc.tensor.matmul(
                    fps, lg[1][:, cc, :], wo_sb[:, 4 + cc, :],
                    start=False, stop=(cc == 3),
                )
            out_sb = o_pool.tile([128, NO], BF16, name="out_sb", tag="out_sb")
            nc.vector.tensor_add(out_sb, fps, bo_bc)
            nc.sync.dma_start(out_ext[a], out_sb)

        # ---------------- phase 0: per-group Q projection ----------------
        # Three hc-blocks (3+3+2) cc-major over a 6-bank pool; both nh
        # halves share each stationary (LDWEIGHTS halved).  The group-first
        # asset's V/K projection blocks are interleaved between phase-0
        # blocks (they use the separate 2-bank proj pool).
        def qproj_block(g, hcs, qps_pool):
            queryT = queryTg_t[g]
            ps = [
                [
                    qps_pool.tile(
                        [128, 512], F32, name=f"qps{x}{k}", tag=f"qps{x}{k}"
                    )
                    for k in range(len(hcs))
                ]
                for x in range(2)
            ]
            for cc in range(4):
                for k, hc in enumerate(hcs):
                    st = wq_sb[:, cc, hc * 128 : (hc + 1) * 128]
                    nc.tensor.matmul(
                        ps[0][k], st, qTg_t[g][:, cc, 0:512],
                        start=(cc == 0), stop=(cc == 3),
                    )
                    nc.tensor.matmul(
                        ps[1][k], st, qTg_t[g][:, cc, 512:1024],
                        start=(cc == 0), stop=(cc == 3),
                    )
            for k, hc in enumerate(hcs):
                if k % 2 == 0:
                    nc.vector.tensor_copy(queryT[:, hc, 0:512], ps[0][k])
                    nc.scalar.copy(queryT[:, hc, 512:1024], ps[1][k])
                else:
                    nc.scalar.copy(queryT[:, hc, 0:512], ps[0][k])
                    nc.vector.tensor_copy(queryT[:, hc, 512:1024], ps[1][k])

        def emit_phase0(g):
            a0 = g * B_G
            queryTg_t[g] = grp_pool.tile(
                [128, 8, GW], BF16, name="queryTg", tag="queryTg"
            )
            ph0 = ExitStack()
            qps_pool = ph0.enter_context(
                tc.tile_pool(name=f"qps{g}", bufs=1, space="PSUM")
            )
            qproj_block(g, [0, 1, 2], qps_pool)
            emit_vaug_alloc(a0)
            for jc in range(4):
                for nh in range(2):
                    ev = emit_vproj_half(a0, jc, nh)
                    if ev is not None:
                        ev()
            qproj_block(g, [3, 4, 5], qps_pool)
            for hc in range(4):
                ev = emit_kproj_block(a0, hc)
                if ev is not None:
                    ev()
            qproj_block(g, [6, 7], qps_pool)
            for hc in range(4, 8):
                ev = emit_kproj_block(a0, hc)
                if ev is not None:
                    ev()
            ph0.close()

        # ---------------- attention slot pipeline ------------------------
        P = {}  # current group's PSUM pools

        def emit_scores2(a, t):
            # head pair (2t, 2t+1): head-pair tile hc == t, hp == head % 2
            kT = kT_tiles[a]
            queryT = queryTg_t[a // B_G]
            ai = a % B_G
            sps = P["score"].tile([128, 2, 4, I], F32, name="sps", tag="sps")
            for e in range(2):
                for jc in range(4):
                    nc.tensor.matmul(
                        sps[:, e, jc, :],
                        kT[e * D : (e + 1) * D, t, jc * 128 : (jc + 1) * 128],
                        queryT[e * D : (e + 1) * D, t, ai * I : (ai + 1) * I],
                        start=True,
                        stop=True,
                    )
            return sps

        def emit_exp_pv2(a, t, sps, aug):
            vaug = vaug_tiles[a]
            expT = exp_pool.tile([128, 2, 4, I], BF16, name="expT", tag="expT")
            nc.scalar.activation(
                expT,
                sps,
                mybir.ActivationFunctionType.Exp,
                bias=0.0,
                scale=0.125,
            )
            for e in range(2):
                h = 2 * t + e
                for jc in range(4):
                    nc.tensor.matmul(
                        aug[:, 2 * (t % 2) + e, :],
                        vaug[:, jc, h, :],
                        expT[:, e, jc, :],
                        start=(jc == 0),
                        stop=(jc == 3),
                    )

        def emit_normalize(a, hg, aug):
            # 1/s: ScalarE copies the PSUM s row (partition 64) down to
            # partition 0, approx reciprocal on DVE (custom uops want base
            # partition 0), broadcast across partitions, three multiplies
            srow = s_pool.tile([1, 4 * I], F32, name="srow", tag="srow")
            if hg % 2 == 0:
                nc.scalar.copy(srow, aug[D : D + 1, :, :])
            else:
                nc.vector.tensor_copy(srow, aug[D : D + 1, :, :])
            rrow = s_pool.tile([1, 4 * I], F32, name="rrow", tag="rrow")
            nc.vector.reciprocal_approx_fast(rrow, srow)
            rb = rb_pool.tile([128, 4, I], F32, name="rb", tag="rb")
            nc.gpsimd.partition_broadcast(rb[:], rrow[:])

            if hg == 0:
                lgs[a] = [
                    l_pool.tile([128, 4, I], BF16, name=f"lg{x}", tag=f"lg{x}")
                    for x in range(2)
                ]
            g, half = hg // 2, hg % 2
            lg = lgs[a][g][:, 2 * half : 2 * half + 2, :]
            nc.vector.tensor_mul(
                lg[0:64, :, :], aug[0:64, 0:4:2, :], rb[0:64, 0:4:2, :]
            )
            nc.vector.tensor_mul(
                lg[64:128, :, :], aug[0:64, 1:4:2, :], rb[0:64, 1:4:2, :]
            )

        # Slot fillers: projections for asset a+1 (same group only; the
        # group-first asset's projections run in its group's phase 0) and
        # the final projection of asset a-1:
        #   t0: vaug alloc + V(jc0) halves       t4: V(jc2) halves
        #   t1: K(a, hc4-5)                      t5: V(jc3) halves + K(x, 0)
        #   t2: K(a, hc6-7)                      t6: K(x, 1-2)
        #   t3: V(jc1) halves                    t7: K(x, 3) + final(a-1)
        def filler_list(a, t):
            ai = a % B_G
            x = a + 1
            xin = (x % B_G != 0) and (x < TOT)  # x in same group
            fl = []
            if t == 0 and xin:
                fl = [
                    lambda: emit_vaug_alloc(x),
                    lambda: emit_vproj_half(x, 0, 0),
                    lambda: emit_vproj_half(x, 0, 1),
                ]
            elif t == 1 and ai >= 1:
                fl = [
                    lambda: emit_kproj_block(a, 4),
                    lambda: emit_kproj_block(a, 5),
                ]
            elif t == 2 and ai >= 1:
                fl = [
                    lambda: emit_kproj_block(a, 6),
                    lambda: emit_kproj_block(a, 7),
                ]
            elif t in (3, 4) and xin:
                jc = t - 2
                fl = [
                    (lambda: emit_vproj_half(x, jc, 0)),
                    (lambda: emit_vproj_half(x, jc, 1)),
                ]
            elif t == 5 and xin:
                fl = [
                    lambda: emit_vproj_half(x, 3, 0),
                    lambda: emit_vproj_half(x, 3, 1),
                    lambda: emit_kproj_block(x, 0),
                ]
            elif t == 6 and xin:
                fl = [
                    lambda: emit_kproj_block(x, 1),
                    lambda: emit_kproj_block(x, 2),
                ]
            elif t == 7:
                fl = []
                if xin:
                    fl.append(lambda: emit_kproj_block(x, 3))
                if ai >= 1:
                    fl.append(lambda: emit_final(a - 1))
            return fl

        augs = {}

        def get_aug(a, t):
            hg = t // 2
            if (a, hg) not in augs:
                augs[(a, hg)] = P["aug"].tile(
                    [D + 1, 4, I], F32, name="aug", tag="aug"
                )
            return augs[(a, hg)]

        for g in range(G):
            emit_phase0(g)
            sc = ExitStack()
            P["score"] = sc.enter_context(
                tc.tile_pool(name=f"score_ps{g}", bufs=2, space="PSUM")
            )
            P["aug"] = sc.enter_context(
                tc.tile_pool(name=f"aug_ps{g}", bufs=2, space="PSUM")
            )
            if g + 1 < G:
                prefetch_group(g + 1)

            prev = None
            deferred = []
            for a in range(g * B_G, (g + 1) * B_G):
                for t in range(8):
                    cur = emit_scores2(a, t)
                    evs = []
                    for f in filler_list(a, t):
                        ev = f()
                        if ev is not None:
                            evs.append(ev)
                    if prev is not None:
                        pa, pt, psps = prev
                        emit_exp_pv2(pa, pt, psps, get_aug(pa, pt))
                        for ev in deferred:
                            ev()
                        if pt % 2 == 1:
                            emit_normalize(pa, pt // 2, augs.pop((pa, pt // 2)))
                    deferred = evs
                    prev = (a, t, cur)
            # group epilogue: flush the exp/normalize pipeline and the
            # group-last asset's final projection
            pa, pt, psps = prev
            emit_exp_pv2(pa, pt, psps, get_aug(pa, pt))
            for ev in deferred:
                ev()
            emit_normalize(pa, 3, augs.pop((pa, 3)))
            emit_final(pa)
            sc.close()

    nc.compile()
    return nc


def make_in_maps(q, kv, Wq, Wkv, Wo, bo):
    import ml_dtypes

    bf = ml_dtypes.bfloat16
    q = np.asarray(q, dtype=np.float32).astype(bf)
    kv = np.asarray(kv, dtype=np.float32).astype(bf)
    Wqb = np.ascontiguousarray(np.asarray(Wq, dtype=np.float32).astype(bf))
    Wkvb = np.ascontiguousarray(np.asarray(Wkv, dtype=np.float32).astype(bf))
    Wob = np.ascontiguousarray(np.asarray(Wo, dtype=np.float32).astype(bf))
    bo = np.ascontiguousarray(np.asarray(bo, dtype=np.float32))

    in_maps = []
    for c in range(N_CORES):
        sl = slice(c * TOT, (c + 1) * TOT)
        qT = np.ascontiguousarray(
            q[sl].transpose(2, 0, 1).reshape(QD, TOT * I)
        )
        kvT = np.ascontiguousarray(kv[sl].transpose(0, 2, 1))
        in_maps.append(
            {"qT": qT, "kvT": kvT, "Wq": Wqb, "Wkv": Wkvb, "Wo": Wob, "bo": bo}
        )
    return in_maps


_CACHED_NC = None


def kernel(q, kv, Wq, Wkv, Wo, bo):
    global _CACHED_NC
    from concourse.bass_utils import run_bass_kernel_spmd

    if _CACHED_NC is None:
        _CACHED_NC = build_nc()
    nc = _CACHED_NC

    in_maps = make_in_maps(q, kv, Wq, Wkv, Wo, bo)
    res = run_bass_kernel_spmd(nc, in_maps, list(range(N_CORES)))
    out = np.concatenate(
        [
            np.asarray(res.results[c]["out"], dtype=np.float32).reshape(
                TOT, I, NO
            )
            for c in range(N_CORES)
        ],
        axis=0,
    )
    return out
